# revision 25
# baseline (speedup 1.0000x reference)
"""Trainium2 8-core kernel for nn_AnalyticFlow (retrieval_knn) — fused fp8 stream.

Math (reference):
    z[b,p] = alpha_b * (x_b . g_p)      (softmax logits; the per-row quadratic
             term is dropped exactly, and the tn^2*||g||^2 bias term is dropped
             because its across-p spread is <= inv_var*tn^2*std(||g||^2) ~ 4e-5
             in logit units -- invisible at the 2e-2 gate)
    w      = softmax_p(z)
    out    = (1/(1-tn_b)) * (sum_p w[b,p] g_p - x_b)

Device strategy (SPMD over 8 NeuronCores, P sharded 6250/core):
    Single fused pipeline over p-windows of 512 (last window 128-wide):
      - mm1: z[b, pwin] = x''^T.T @ G'^T (fp8 DoubleRowSwInterleave, x''
        stationary, 2^9 prescale keeps fp8 out of the subnormal range)
      - ScalarE exp(scale=1/XSCALE) -> E fp8 + softmax denominator via
        accum_out (the partial last window is split so pads never hit s)
      - PE transposes E 128x128 tiles (batched per window, prev-window
        deferred so PE never waits on ScalarE)
      - mm2 for the window's pairs is interleaved into the SAME loop two
        windows behind: W[b, dg] accumulates 2 p-pairs in one PSUM bank
        (fp8 DoubleRow), then VectorE flushes psum into the resident
        f32 W accumulator.  This keeps <= 7 PSUM banks live and lets the
        gN DMA stream fully overlap mm1 instead of serializing phases.
    Tail: W + s are converted to fp16 and a single ReduceScatter over
    [B, D+1] reduces across cores; each core epilogues its own 32 rows
    (out ~= -x/(1-tn) dominates, so fp16 collectives cost ~1e-4 rel err).
"""

import math

import numpy as np

import concourse.bacc as bacc
import concourse.tile as tile
import concourse.mybir as mybir
from concourse import bass_utils

FP8 = mybir.dt.float8e4
F16 = mybir.dt.float16
F32 = mybir.dt.float32
NP_FP8 = mybir.dt.np(FP8)

T_SCHEDULE = 999.0
N_CORES = 8
XSCALE = 512.0  # power-of-2 prescale on x'' so fp8 values stay normal-range
PW = 512        # p-window width for mm1


class Cfg:
    def __init__(self, B=256, D=3072, P=50000):
        assert B % 128 == 0 and B % N_CORES == 0
        assert D % 512 == 0
        assert P % N_CORES == 0
        self.B = B
        self.D = D
        self.P = P
        self.PSH = P // N_CORES                       # db rows per core
        self.NWF = self.PSH // PW                     # full 512-wide windows
        rem = self.PSH - self.NWF * PW
        self.LW = ((rem + 127) // 128) * 128          # last-window padded width
        self.NW = self.NWF + (1 if rem else 0)        # total windows
        self.PCH = math.ceil(self.PSH / 256)          # p-pairs for mm2
        self.KCH = D // 256                           # K-pairs (d-chunks / 2)
        self.NDG = D // 512                           # 512-wide d-groups
        self.BC = B // 128                            # b-chunks
        self.BR = B // N_CORES                        # output rows per core
        self.FOLD = 128 // self.BR                    # epilogue row-fold
        self.EPL = D // self.FOLD                     # epilogue cols/partition
        assert D % self.FOLD == 0
        assert self.NW >= 3


def build_nc(cfg: Cfg):
    nc = bacc.Bacc(
        "TRN2", target_bir_lowering=False, debug=False, num_devices=N_CORES
    )
    gT = nc.declare_dram_parameter(
        "gT", [cfg.NWF, 128, cfg.KCH, 2, PW], FP8, isOutput=False
    )
    gTl = nc.declare_dram_parameter(
        "gTl", [128, cfg.KCH, 2, max(cfg.LW, 128)], FP8, isOutput=False
    )
    gN = nc.declare_dram_parameter(
        "gN", [cfg.PCH, 128, 2, cfg.D], FP8, isOutput=False
    )
    # mm1 stationary operand, SW-interleaved for DoubleRowSwInterleave:
    # [i, k, m, j] = x''T[(2k+j)*128+i, bchunk*128 + (127-m)]
    xT = nc.declare_dram_parameter(
        "xT", [128, cfg.KCH, cfg.B, 2], FP8, isOutput=False
    )
    xi = nc.declare_dram_parameter("xi", [128, cfg.EPL], F32, isOutput=False)
    sc = nc.declare_dram_parameter("sc", [128, 1], F32, isOutput=False)
    out = nc.declare_dram_parameter("out", [cfg.BR, cfg.D], F32, isOutput=True)

    DR = mybir.MatmulPerfMode.DoubleRow
    DRSW = mybir.MatmulPerfMode.DoubleRowSwInterleave
    EXP = mybir.ActivationFunctionType.Exp

    ident_np = np.eye(128, dtype=NP_FP8)
    ident_dram = nc.inline_tensor(ident_np, name="ident")

    with tile.TileContext(nc) as tc:
        with (
            tc.tile_pool(name="persist", bufs=1) as pp,
            tc.tile_pool(name="gtp", bufs=4) as gtp,
            tc.tile_pool(name="gnp", bufs=7) as gnp,
            tc.tile_pool(name="ep", bufs=4) as epool,
            tc.tile_pool(name="stg", bufs=1) as stg,
            tc.tile_pool(name="dram", bufs=1, space="DRAM") as dram,
            tc.tile_pool(name="zps", bufs=2, space="PSUM") as zpool,
            tc.tile_pool(name="wps", bufs=3, space="PSUM") as wpool,
            tc.tile_pool(name="tps", bufs=2, space="PSUM") as tpool,
        ):
            # ---- preamble: prioritized DMA, k-chunked so the first mm1
            # accumulation chain starts as soon as its k-slices land ----
            gt_tiles = {}
            gt0 = gtp.tile([128, cfg.KCH, 2, PW], FP8, tag="gt", name="gt0")
            xT_sb = pp.tile([128, cfg.KCH, cfg.B, 2], FP8)
            kc = max(1, cfg.KCH // 6)
            cuts = list(range(0, cfg.KCH, kc)) + [cfg.KCH]
            for a, b_ in zip(cuts[:-1], cuts[1:]):
                nc.sync.dma_start(gt0[:, a:b_, :, :], gT[0, :, a:b_, :, :])
                nc.sync.dma_start(xT_sb[:, a:b_, :, :], xT[:, a:b_, :, :])
            gt_tiles[0] = gt0
            ident_sb = pp.tile([128, 128], FP8)
            nc.sync.dma_start(ident_sb[:], ident_dram[:])
            xi_sb = pp.tile([128, cfg.EPL], F32)
            nc.sync.dma_start(xi_sb[:], xi[:])
            sc_sb = pp.tile([128, 1], F32)
            nc.sync.dma_start(sc_sb[:], sc[:])

            ET = pp.tile([128, cfg.PCH, 2, cfg.B], FP8)
            s_acc = [
                pp.tile([128, 1], F32, name=f"sacc{b}", tag=f"sacc{b}")
                for b in range(cfg.BC)
            ]
            Wah = [
                pp.tile([128, cfg.D], F32, name=f"wah{b}", tag=f"wah{b}")
                for b in range(cfg.BC)
            ]
            for b in range(cfg.BC):
                nc.vector.memset(s_acc[b][:], 0.0)
            # pair tail not covered by any window's transposes (K pad rows)
            cov = (cfg.NWF * PW + cfg.LW)  # p rows written by transposes
            if cov < cfg.PCH * 256:
                pad0 = cov - (cfg.PCH - 1) * 256  # valid rows in last pair
                nc.vector.memset(
                    ET[:, cfg.PCH - 1, pad0 // 128 :, :].rearrange(
                        "p a b -> p (a b)"
                    ),
                    0.0,
                )

            # W travels a SINGLE ReduceScatter in fp16 with s appended as
            # one extra column: every extra collective costs ~8us fixed and
            # they serialize on the cc stream, so one op is strictly best.
            # (fp8 reduce is not supported by the runtime — HW returns inf.)
            RSW = cfg.D + 1
            rs_sb = pp.tile([128, cfg.BC, RSW], F16)
            rs_in = dram.tile([cfg.B, RSW], F16, name="rsin")
            rs_out = dram.tile([cfg.BR, RSW], F16, name="rsout")

            gn_tiles = {}

            def issue_gt(w):
                if w >= cfg.NW or w in gt_tiles:
                    return
                if w < cfg.NWF:
                    t = gtp.tile([128, cfg.KCH, 2, PW], FP8, tag="gt",
                                 name=f"gt{w}")
                    nc.sync.dma_start(t[:], gT[w])
                else:
                    t = gtp.tile([128, cfg.KCH, 2, cfg.LW], FP8, tag="gt",
                                 name=f"gt{w}")
                    nc.sync.dma_start(t[:], gTl[:, :, :, : cfg.LW])
                gt_tiles[w] = t

            def issue_gn(m):
                if m >= cfg.PCH or m in gn_tiles:
                    return
                t = gnp.tile([128, 2, cfg.D], FP8, tag="gn", name=f"gn{m}")
                nc.sync.dma_start(t[:], gN[m])
                gn_tiles[m] = t

            def emit_exp(w, b, z, pww):
                vc = max(0, min(cfg.PSH - w * PW, pww))
                e_t = epool.tile([128, pww], FP8, tag="e", name=f"e{w}_{b}")
                s_part = pp.tile(
                    [128, 1], F32, name=f"sp{w}_{b}", tag="spart", bufs=4
                )
                if vc == pww:
                    nc.scalar.activation(
                        e_t[:], z[:], EXP, scale=1.0 / XSCALE,
                        accum_out=s_part[:],
                    )
                else:
                    nc.scalar.activation(
                        e_t[:, :vc], z[:, :vc], EXP, scale=1.0 / XSCALE,
                        accum_out=s_part[:],
                    )
                    nc.scalar.activation(
                        e_t[:, vc:], z[:, vc:], EXP, scale=1.0 / XSCALE,
                    )
                nc.vector.tensor_add(s_acc[b][:], s_acc[b][:], s_part[:])
                return e_t

            def emit_tr(w, es, pww):
                # transpose the window's E blocks (both b-chunks) through
                # one PSUM tile, then one batched ScalarE copy per b-chunk
                nt = pww // 128
                t_ps = tpool.tile([128, cfg.BC * nt * 128, 2], FP8, tag="t",
                                  name=f"t{w}")
                for b in range(cfg.BC):
                    for c in range(nt):
                        nc.tensor.transpose(
                            t_ps[:, (b * nt + c) * 128 : (b * nt + c + 1) * 128, 0],
                            es[b][:, c * 128 : (c + 1) * 128],
                            ident_sb[:],
                        )
                pb = (w * PW) // 256
                npair = (nt + 1) // 2
                for b in range(cfg.BC):
                    src = t_ps[:, b * nt * 128 : (b + 1) * nt * 128, 0].rearrange(
                        "p (a c) -> p a c", c=128
                    )
                    dst = ET[
                        :, pb : pb + npair, :, b * 128 : (b + 1) * 128
                    ].rearrange("p a b c -> p (a b) c")[:, :nt, :]
                    nc.scalar.copy(dst, src)

            def emit_block(pairs, first, final=False):
                for dg in range(cfg.NDG):
                    c0 = dg * 512
                    for b in range(cfg.BC):
                        acc = wpool.tile([128, 512], F32, tag="w",
                                         name=f"acc{pairs[0]}_{dg}_{b}")
                        for j, m in enumerate(pairs):
                            nc.tensor.matmul(
                                acc[:],
                                ET[:, m, :, b * 128 : (b + 1) * 128],
                                gn_tiles[m][:, :, c0 : c0 + 512],
                                start=(j == 0),
                                stop=(j == len(pairs) - 1),
                                perf_mode=DR,
                            )
                        dst = Wah[b][:, c0 : c0 + 512]
                        if final:
                            # last contribution: fold the fp16 downconvert
                            # into the flush and ship this RS slice now
                            rdst = rs_sb[:, b, c0 : c0 + 512]
                            nc.vector.tensor_add(rdst, dst, acc[:])
                            nc.sync.dma_start(
                                rs_in[b * 128 : (b + 1) * 128, c0 : c0 + 512],
                                rdst,
                            )
                        elif first:
                            nc.scalar.copy(dst, acc[:])
                        else:
                            nc.vector.tensor_add(dst, dst, acc[:])

            # ---------------- fused main loop ----------------
            pending_tr = None
            pair_cur = 0
            issue_gt(1)  # stay two gT windows ahead: the mm1-only ramp-up
            for w in range(cfg.NW):  # windows eat gT faster than steady state
                issue_gt(w + 2)
                issue_gn(2 * w)
                issue_gn(2 * w + 1)
                pww = PW if w < cfg.NWF else cfg.LW
                gt = gt_tiles.pop(w)
                es = []
                for b in range(cfg.BC):
                    z = zpool.tile([128, pww], F32, tag="z", name=f"z{w}_{b}")
                    for k in range(cfg.KCH):
                        nc.tensor.matmul(
                            z[:],
                            xT_sb[:, k, b * 128 : (b + 1) * 128, :],
                            gt[:, k, :, :],
                            start=(k == 0),
                            stop=(k == cfg.KCH - 1),
                            perf_mode=DRSW,
                        )
                    es.append(emit_exp(w, b, z, pww))
                if pending_tr is not None:
                    emit_tr(*pending_tr)
                pending_tr = (w, es, pww)
                # mm2 for pairs two windows behind (their ET copies landed
                # during the previous window)
                hi = min(2 * (w - 1), cfg.PCH) if w >= 2 else 0
                while pair_cur < hi:
                    blk = list(range(pair_cur, min(pair_cur + 2, hi)))
                    emit_block(blk, first=(pair_cur == 0))
                    pair_cur += len(blk)

            # ---------------- drain ----------------
            # s is final after the last window: convert + ship its column
            # so the RS is gated only by the drain's W slices
            for b in range(cfg.BC):
                nc.scalar.copy(rs_sb[:, b, cfg.D : RSW], s_acc[b][:])
                nc.sync.dma_start(
                    rs_in[b * 128 : (b + 1) * 128, cfg.D : RSW],
                    rs_sb[:, b, cfg.D : RSW],
                )
            if pending_tr is not None:
                emit_tr(*pending_tr)
            blk = list(range(pair_cur, cfg.PCH))
            emit_block(blk, first=False, final=True)

            nc.gpsimd.collective_compute(
                "ReduceScatter",
                mybir.AluOpType.add,
                replica_groups=[list(range(N_CORES))],
                ins=[rs_in.opt()],
                outs=[rs_out.opt()],
            )

            # ------------- epilogue (folded to 128 partitions) -------------
            # [BR, D] reshaped as [128, EPL]: partition BR*c + b holds row b,
            # cols [c*EPL, (c+1)*EPL) — xi/sc are host-prepped to match.
            FOLD, EPL = cfg.FOLD, cfg.EPL
            out_sb = pp.tile([128, EPL], F32)
            rec = pp.tile([128, 1], F32)
            s_f = stg.tile([128, 1], F16, name="sph")
            eph = stg.tile([128, EPL], F16, name="eph")
            for c in range(FOLD):
                nc.sync.dma_start(
                    s_f[c * cfg.BR : (c + 1) * cfg.BR, :],
                    rs_out[:, cfg.D : RSW],
                )
                nc.sync.dma_start(
                    eph[c * cfg.BR : (c + 1) * cfg.BR, :],
                    rs_out[:, c * EPL : (c + 1) * EPL],
                )
            nc.vector.reciprocal(rec[:], s_f[:])
            nc.vector.tensor_mul(rec[:], rec[:], sc_sb[:])
            nc.vector.scalar_tensor_tensor(
                out_sb[:],
                eph[:],
                rec[:],
                xi_sb[:],
                op0=mybir.AluOpType.mult,
                op1=mybir.AluOpType.subtract,
            )
            for c in range(FOLD):
                nc.sync.dma_start(
                    out[:, c * EPL : (c + 1) * EPL],
                    out_sb[c * cfg.BR : (c + 1) * cfg.BR, :],
                )

    nc.compile()
    return nc


def prep_in_maps(cfg: Cfg, xt, t, gt_images):
    B, D, P = cfg.B, cfg.D, cfg.P
    x = np.asarray(xt, dtype=np.float32).reshape(B, -1)
    g = np.asarray(gt_images, dtype=np.float32).reshape(P, -1)
    t = np.asarray(t, dtype=np.float32).reshape(B)
    assert x.shape[1] == D

    tn = t / T_SCHEDULE
    inv_var = 1.0 / (2.0 * (1.0 - tn) ** 2)
    alpha = 2.0 * inv_var * tn
    inv1mtn = 1.0 / (1.0 - tn)

    # x''^T pretiled + SW-interleaved for DoubleRowSwInterleave:
    # [128, KCH, B, 2]: [i, k, bc*128+m, j] = x''T[(2k+j)*128+i, bc*128+127-m]
    xp = (x * (alpha * XSCALE)[:, None]).T  # [D, B]
    x4 = xp.reshape(cfg.KCH, 2, 128, B // 128, 128)   # [k, j, i, bc, n]
    x4 = x4[:, :, :, :, ::-1]                          # reverse cols in chunk
    xT_tiled = np.ascontiguousarray(
        x4.transpose(2, 0, 3, 4, 1).reshape(128, cfg.KCH, B, 2)
    ).astype(NP_FP8)

    LWD = max(cfg.LW, 128)
    PWTOT = cfg.NWF * PW + LWD
    in_maps = []
    for c in range(N_CORES):
        gs = g[c * cfg.PSH : (c + 1) * cfg.PSH]
        # G'^T padded [D, PWTOT] -> full windows [NWF, 128, KCH, 2, PW]
        # plus last window [128, KCH, 2, LWD]
        gtp_ = np.zeros((D, PWTOT), np.float32)
        gtp_[:, : cfg.PSH] = gs.T
        g5 = gtp_.reshape(cfg.KCH, 2, 128, PWTOT)
        gTb = np.ascontiguousarray(
            g5[:, :, :, : cfg.NWF * PW]
            .reshape(cfg.KCH, 2, 128, cfg.NWF, PW)
            .transpose(3, 2, 0, 1, 4)
        ).astype(NP_FP8)
        gTlb = np.ascontiguousarray(
            g5[:, :, :, cfg.NWF * PW :].transpose(2, 0, 1, 3)
        ).astype(NP_FP8)
        # G_nat pair tiles [PCH, 128, 2, D]: [m, i, j, d] = G[(2m+j)*128+i, d]
        # prescaled 1/4 so W partials ride the ReduceScatter in fp8
        gn_ = np.zeros((cfg.PCH * 256, cfg.D), np.float32)
        gn_[: cfg.PSH] = gs * 0.25
        gNb = np.ascontiguousarray(
            gn_.reshape(cfg.PCH, 2, 128, cfg.D).transpose(0, 2, 1, 3)
        ).astype(NP_FP8)
        rows = slice(c * cfg.BR, (c + 1) * cfg.BR)
        # epilogue operands folded to [128, EPL]: partition BR*f + b holds
        # output row b, columns [f*EPL, (f+1)*EPL)
        xi_full = x[rows] * inv1mtn[rows, None]
        xi = np.ascontiguousarray(
            xi_full.reshape(cfg.BR, cfg.FOLD, cfg.EPL)
            .transpose(1, 0, 2)
            .reshape(128, cfg.EPL)
        ).astype(np.float32)
        # 4x undoes the gN prescale
        sc = np.ascontiguousarray(
            np.tile(4.0 * inv1mtn[rows, None], (cfg.FOLD, 1))
        ).astype(np.float32)
        in_maps.append(
            {"gT": gTb, "gTl": gTlb, "gN": gNb, "xT": xT_tiled, "xi": xi, "sc": sc}
        )
    return in_maps


_NC_CACHE = {}


def _get_nc(cfg: Cfg):
    key = (cfg.B, cfg.D, cfg.P)
    if key not in _NC_CACHE:
        _NC_CACHE[key] = build_nc(cfg)
    return _NC_CACHE[key]


def kernel(xt, t, gt_images, _trace=False):
    xt = np.asarray(xt)
    cfg = Cfg(B=xt.shape[0], D=int(np.prod(xt.shape[1:])),
              P=np.asarray(gt_images).shape[0])
    nc = _get_nc(cfg)
    in_maps = prep_in_maps(cfg, xt, t, gt_images)
    res = bass_utils.run_bass_kernel_spmd(
        nc, in_maps, core_ids=list(range(N_CORES)), trace=_trace
    )
    out = np.concatenate(
        [res.results[c]["out"] for c in range(N_CORES)], axis=0
    ).astype(np.float32)
    if _trace:
        kernel.last_exec_time_ns = res.exec_time_ns
        kernel.last_result = res
    return out.reshape(xt.shape)
